# revision 10
# baseline (speedup 1.0000x reference)
"""Trainium2 Bass kernel for the MemoryModule problem.

Computes, for query [N, 64] and memory [1024, 64]:
    out = softmax(query @ memory.T, axis=-1) @ memory

Data-parallel across 8 NeuronCores: query sharded along N, memory
replicated.  Per-core plan (N_loc = 32768 rows, slabs of 512):

  1. DMA q slab [512, 64] -> SBUF, PE-transpose to qT [64, 512],
     replicated into both partition halves [128, 512] (f32r).  The qT
     chain for slab k+1 is issued at the top of slab k so it hides under
     slab k's exps (the ACT engine is the kernel's bottleneck).
  2. mm1 (f32r, 2x row-tiled): memory-chunk pairs run concurrently in
     the two 64-row halves of the PE array (K=64), producing S^T chunks
     [128, 512] in PSUM (3 bank-groups of <=3 chunks, double buffered).
  3. One Exp activation per group, PSUM -> SBUF bf16 (A^T), unnormalized.
  4. mm2 (bf16): U^T [65, 512] += [mem_c | 1].T @ A^T[c], accumulated over
     all 8 chunks in one PSUM bank.  Row 64 is the softmax denominator.
  5. PE-transpose U^T back to [n, d] chunks, DVE reciprocal + scale,
     DMA out.

The softmax skips max-subtraction: |S| <= |q||m| <= ~45 for these inputs
(randn, D=64), exp fits comfortably in fp32/bf16 range.
"""

import numpy as np

import concourse.bass as bass
import concourse.tile as tile
from concourse import bacc, mybir
from concourse.bass_utils import run_bass_kernel_spmd
from concourse.masks import make_identity

N, M, D = 262144, 1024, 64
NCORES = 8
NLOC = N // NCORES
SLAB = 512
NT = SLAB // 128  # 128-row tiles per slab
MCHUNKS = M // 128  # 8
GROUPS = [(0, 3), (3, 3), (6, 2)]  # (first chunk, n chunks) per exp group
ROWTILE = False  # run memory-chunk pairs concurrently in PE row halves

F32 = mybir.dt.float32
F32R = mybir.dt.float32r
BF16 = mybir.dt.bfloat16
EXP = mybir.ActivationFunctionType.Exp


def _chunk_slot(c):
    for g, (c0, cn) in enumerate(GROUPS):
        if c0 <= c < c0 + cn:
            return g, c - c0
    raise ValueError(c)


def _body(tc: tile.TileContext, out: bass.AP, q: bass.AP, mem: bass.AP, repeat: int):
    nc = tc.nc
    n_loc = q.shape[0]
    nslabs = n_loc // SLAB

    with (
        tc.tile_pool(name="const", bufs=1) as const,
        tc.tile_pool(name="qpool", bufs=2) as qpool,
        tc.tile_pool(name="qtpool", bufs=2) as qtpool,
        tc.tile_pool(name="atpool", bufs=2) as atpool,
        tc.tile_pool(name="uspool", bufs=2) as uspool,
        tc.tile_pool(name="rpool", bufs=2) as rpool,
        tc.tile_pool(name="opool", bufs=3) as opool,
    ):
        ident = const.tile([128, 128], F32, tag="ident")
        make_identity(nc, ident)

        # touch Exp immediately so the ACT table load overlaps the preamble
        warm = const.tile([128, 2], F32, tag="warm")
        nc.vector.memset(warm, 0.0)
        nc.scalar.activation(warm[:, 1:2], warm[:, 0:1], EXP)

        # memory staged [128, chunk, d]; row m = 128*c + p
        mstage = const.tile([128, MCHUNKS, D], F32, tag="mstage")
        nc.sync.dma_start(out=mstage, in_=mem.rearrange("(c p) d -> p c d", p=128))

        # mm2 stationary: [mem_c | ones] in bf16
        maug = const.tile([128, MCHUNKS, D + 1], BF16, tag="maug")
        nc.vector.tensor_copy(maug[:, :, 0:D], mstage)
        nc.vector.memset(maug[:, :, D : D + 1], 1.0)

        # memT [d, m] via PE transposes (one-time PSUM staging pool).
        # Stored as float32r, replicated into both partition halves so
        # chunk pairs can run row-tiled in the two halves of the PE array.
        memT_r = const.tile([128, M], F32R, tag="memT")
        with tc.tile_pool(name="mtps", bufs=1, space="PSUM") as mtps:
            mt_ps = mtps.tile([64, M], F32, tag="mtps")
            for c in range(MCHUNKS):
                nc.tensor.transpose(
                    mt_ps[:, 128 * c : 128 * (c + 1)], mstage[:, c, :], ident
                )
            nc.vector.tensor_copy(memT_r[0:64, :], mt_ps)
            nc.vector.tensor_copy(memT_r[64:128, :], mt_ps)

        qv = q.rearrange("(s t p) d -> s p t d", p=128, t=NT)
        ov = out.rearrange("(s t p) d -> s p t d", p=128, t=NT)

        with (
            tc.tile_pool(name="spsum", bufs=2, space="PSUM") as spsum,
            tc.tile_pool(name="qtpsum", bufs=1, space="PSUM") as qtpsum,
            tc.tile_pool(name="upsum", bufs=1, space="PSUM") as upsum,
        ):

            def load_qt(i):
                """DMA slab i's q and produce replicated qT [128, SLAB] f32r."""
                q_sb = qpool.tile([128, NT, D], F32, tag="q")
                nc.sync.dma_start(out=q_sb, in_=qv[i])
                qt_ps = qtpsum.tile([64, SLAB], F32, tag="qt")
                for t in range(NT):
                    nc.tensor.transpose(
                        qt_ps[:, 128 * t : 128 * (t + 1)], q_sb[:, t, :], ident
                    )
                qt_r = qtpool.tile([128, SLAB], F32R, tag="qts")
                nc.vector.tensor_copy(qt_r[0:64, :], qt_ps)
                nc.vector.tensor_copy(qt_r[64:128, :], qt_ps)
                return qt_r

            def new_slab(k):
                return {
                    "at": atpool.tile([128, MCHUNKS, SLAB], BF16, tag="at", name=f"at{k}"),
                    "u": upsum.tile([65, SLAB], F32, tag="u", name=f"u{k}"),
                    "s": [
                        spsum.tile([128, 3, SLAB], F32, tag="s", name=f"s{k}_{g}")
                        for g in range(len(GROUPS))
                    ],
                }

            def mm1(st, qt, chunks, paired):
                for h, c in enumerate(chunks):
                    g, j = _chunk_slot(c)
                    lo = 64 * h if (paired and ROWTILE) else 0
                    nc.tensor.matmul(
                        st["s"][g][:, j, :],
                        lhsT=memT_r[lo : lo + 64, 128 * c : 128 * (c + 1)],
                        rhs=qt[lo : lo + 64, :],
                        start=True,
                        stop=True,
                        tile_position=(lo, 0) if ROWTILE else None,
                    )

            def exp_group(st, g):
                c0, cn = GROUPS[g]
                nc.scalar.activation(
                    st["at"][:, c0 : c0 + cn, :], st["s"][g][:, 0:cn, :], EXP
                )

            def mm2_group(st, g):
                c0, cn = GROUPS[g]
                for j in range(cn):
                    c = c0 + j
                    nc.tensor.matmul(
                        st["u"],
                        lhsT=maug[:, c, :],
                        rhs=st["at"][:, c, :],
                        start=(c == 0),
                        stop=(c == MCHUNKS - 1),
                    )

            def epilogue(st, i):
                u_sb = uspool.tile([65, SLAB], F32, tag="us", name=f"us{i}")
                nc.vector.tensor_copy(u_sb, st["u"])
                # o_ps shares the qt staging bank (disjoint lifetimes)
                o_ps = qtpsum.tile([128, NT, D + 1], F32, tag="qt", name=f"o{i}")
                for t in range(NT):
                    nc.tensor.transpose(
                        o_ps[:, t, :],
                        u_sb[:, 128 * t : 128 * (t + 1)],
                        ident[0:65, 0:65],
                    )
                r_sb = rpool.tile([128, NT], F32, tag="r", name=f"r{i}")
                nc.vector.reciprocal(r_sb, o_ps[:, :, D])
                o_sb = opool.tile([128, NT, D], F32, tag="o", name=f"o_sb{i}")
                for t in range(NT):
                    nc.vector.tensor_scalar_mul(
                        o_sb[:, t, :], o_ps[:, t, 0:D], r_sb[:, t : t + 1]
                    )
                nc.sync.dma_start(out=ov[i], in_=o_sb)

            def full_pass():
                # Software pipeline: slab k's group-C matmul-2 and epilogue are
                # emitted inside iteration k+1, after slab k+1's first mm1
                # group, so the in-order PE queue never puts exp-blocked work
                # in front of the matmuls the ACT engine is waiting on.
                qt = {0: load_qt(0)}
                st = {}
                for k in range(nslabs):
                    st[k] = new_slab(k)
                    # group-A mm1: pair (0,1) early, chunk 2 solo
                    mm1(st[k], qt[k], (0, 1), paired=True)
                    mm1(st[k], qt[k], (2,), paired=False)
                    if k + 1 < nslabs:
                        qt[k + 1] = load_qt(k + 1)
                    if k > 0:
                        mm2_group(st[k - 1], 2)
                        epilogue(st[k - 1], k - 1)
                        del st[k - 1]
                    exp_group(st[k], 0)
                    mm2_group(st[k], 0)
                    # group-B mm1: chunk 3 solo (waits exp-C(k-1) bank),
                    # then pair (4,5); group-C pair (6,7) right after
                    mm1(st[k], qt[k], (3,), paired=False)
                    mm1(st[k], qt[k], (4, 5), paired=True)
                    mm1(st[k], qt[k], (6, 7), paired=True)
                    exp_group(st[k], 1)
                    mm2_group(st[k], 1)
                    exp_group(st[k], 2)
                    del qt[k]
                last = nslabs - 1
                mm2_group(st[last], 2)
                epilogue(st[last], last)

            if repeat > 1:
                with tc.For_i(0, repeat, 1):
                    full_pass()
            else:
                full_pass()


def build_bass(n_loc: int = NLOC, repeat: int = 1) -> bacc.Bacc:
    nc = bacc.Bacc("TRN2", target_bir_lowering=False, debug=False)
    q = nc.dram_tensor("query", [n_loc, D], F32, kind="ExternalInput").ap()
    mem = nc.dram_tensor("memory", [M, D], F32, kind="ExternalInput").ap()
    out = nc.dram_tensor("out", [n_loc, D], F32, kind="ExternalOutput").ap()
    with tile.TileContext(nc) as tc:
        _body(tc, out, q, mem, repeat)
    nc.compile()
    return nc


_NC_CACHE: dict[tuple[int, int], bacc.Bacc] = {}


def _get_nc(n_loc: int = NLOC, repeat: int = 1) -> bacc.Bacc:
    key = (n_loc, repeat)
    if key not in _NC_CACHE:
        _NC_CACHE[key] = build_bass(n_loc, repeat)
    return _NC_CACHE[key]


def run(query: np.ndarray, memory: np.ndarray, repeat: int = 1, **spmd_kwargs):
    """Run on 8 cores; returns (out [N, 64], BassKernelResults)."""
    query = np.ascontiguousarray(np.asarray(query, dtype=np.float32))
    memory = np.ascontiguousarray(np.asarray(memory, dtype=np.float32))
    assert query.shape == (N, D) and memory.shape == (M, D)
    nc = _get_nc(NLOC, repeat)
    in_maps = [
        {"query": query[i * NLOC : (i + 1) * NLOC], "memory": memory}
        for i in range(NCORES)
    ]
    res = run_bass_kernel_spmd(nc, in_maps, list(range(NCORES)), **spmd_kwargs)
    out = np.concatenate([res.results[i]["out"] for i in range(NCORES)], axis=0)
    return out, res


def kernel(query: np.ndarray, memory: np.ndarray) -> np.ndarray:
    out, _ = run(query, memory)
    return out


# revision 11
# speedup vs baseline: 1.1563x; 1.1563x over previous
"""Trainium2 Bass kernel for the MemoryModule problem.

Computes, for query [N, 64] and memory [1024, 64]:
    out = softmax(query @ memory.T, axis=-1) @ memory

Data-parallel across 8 NeuronCores: query sharded along N, memory
replicated.  Per-core plan (N_loc = 32768 rows, slabs of 512):

  1. DMA q slab [512, 64] -> SBUF, PE-transpose to qT [64, 512],
     replicated into both partition halves [128, 512] (fp16).  The qT
     chain for slab k+1 is issued at the top of slab k so it hides under
     slab k's exps (the ACT engine is the kernel's bottleneck).
  2. mm1 (fp16, 2x row-tiled): memory-chunk pairs run concurrently in
     the two 64-row halves of the PE array (K=64), producing S^T chunks
     [128, 512] in PSUM (3 bank-groups of <=3 chunks, double buffered).
  3. One Exp activation per group, PSUM -> SBUF bf16 (A^T), unnormalized.
  4. mm2 (bf16): U^T [65, 512] += [mem_c | 1].T @ A^T[c], accumulated over
     all 8 chunks in one PSUM bank.  Row 64 is the softmax denominator.
  5. PE-transpose U^T back to [n, d] chunks, DVE reciprocal + scale,
     DMA out.

The softmax skips max-subtraction: |S| <= |q||m| <= ~45 for these inputs
(randn, D=64), exp fits comfortably in fp32/bf16 range.
"""

import numpy as np

import concourse.bass as bass
import concourse.tile as tile
from concourse import bacc, mybir
from concourse.bass_utils import run_bass_kernel_spmd
from concourse.masks import make_identity

N, M, D = 262144, 1024, 64
NCORES = 8
NLOC = N // NCORES
SLAB = 512
NT = SLAB // 128  # 128-row tiles per slab
MCHUNKS = M // 128  # 8
GROUPS = [(0, 3), (3, 3), (6, 2)]  # (first chunk, n chunks) per exp group
ROWTILE = True  # run memory-chunk pairs concurrently in PE row halves

F32 = mybir.dt.float32
F32R = mybir.dt.float32r
F16 = mybir.dt.float16
BF16 = mybir.dt.bfloat16
EXP = mybir.ActivationFunctionType.Exp


def _chunk_slot(c):
    for g, (c0, cn) in enumerate(GROUPS):
        if c0 <= c < c0 + cn:
            return g, c - c0
    raise ValueError(c)


def _body(tc: tile.TileContext, out: bass.AP, q: bass.AP, mem: bass.AP, repeat: int):
    nc = tc.nc
    n_loc = q.shape[0]
    nslabs = n_loc // SLAB

    with (
        tc.tile_pool(name="const", bufs=1) as const,
        tc.tile_pool(name="qpool", bufs=2) as qpool,
        tc.tile_pool(name="qtpool", bufs=2) as qtpool,
        tc.tile_pool(name="atpool", bufs=2) as atpool,
        tc.tile_pool(name="uspool", bufs=2) as uspool,
        tc.tile_pool(name="rpool", bufs=2) as rpool,
        tc.tile_pool(name="opool", bufs=3) as opool,
    ):
        ident = const.tile([128, 128], F32, tag="ident")
        make_identity(nc, ident)

        # touch Exp immediately so the ACT table load overlaps the preamble
        warm = const.tile([128, 2], F32, tag="warm")
        nc.vector.memset(warm, 0.0)
        nc.scalar.activation(warm[:, 1:2], warm[:, 0:1], EXP)

        # memory staged [128, chunk, d]; row m = 128*c + p
        mstage = const.tile([128, MCHUNKS, D], F32, tag="mstage")
        nc.sync.dma_start(out=mstage, in_=mem.rearrange("(c p) d -> p c d", p=128))

        # mm2 stationary: [mem_c | ones] in bf16
        maug = const.tile([128, MCHUNKS, D + 1], BF16, tag="maug")
        nc.vector.tensor_copy(maug[:, :, 0:D], mstage)
        nc.vector.memset(maug[:, :, D : D + 1], 1.0)

        # memT [d, m] via PE transposes (one-time PSUM staging pool).
        # Stored as float32r, replicated into both partition halves so
        # chunk pairs can run row-tiled in the two halves of the PE array.
        memT_r = const.tile([128, M], F16, tag="memT")
        with tc.tile_pool(name="mtps", bufs=1, space="PSUM") as mtps:
            mt_ps = mtps.tile([64, M], F32, tag="mtps")
            for c in range(MCHUNKS):
                nc.tensor.transpose(
                    mt_ps[:, 128 * c : 128 * (c + 1)], mstage[:, c, :], ident
                )
            nc.vector.tensor_copy(memT_r[0:64, :], mt_ps)
            nc.vector.tensor_copy(memT_r[64:128, :], mt_ps)

        qv = q.rearrange("(s t p) d -> s p t d", p=128, t=NT)
        ov = out.rearrange("(s t p) d -> s p t d", p=128, t=NT)

        with (
            tc.tile_pool(name="spsum", bufs=2, space="PSUM") as spsum,
            tc.tile_pool(name="qtpsum", bufs=1, space="PSUM") as qtpsum,
            tc.tile_pool(name="upsum", bufs=1, space="PSUM") as upsum,
        ):

            def load_qt(i):
                """DMA slab i's q and produce replicated qT [128, SLAB] f32r."""
                q_sb = qpool.tile([128, NT, D], F32, tag="q")
                nc.sync.dma_start(out=q_sb, in_=qv[i])
                qt_ps = qtpsum.tile([64, SLAB], F32, tag="qt")
                for t in range(NT):
                    nc.tensor.transpose(
                        qt_ps[:, 128 * t : 128 * (t + 1)], q_sb[:, t, :], ident
                    )
                qt_r = qtpool.tile([128, SLAB], F16, tag="qts")
                nc.vector.tensor_copy(qt_r[0:64, :], qt_ps)
                nc.vector.tensor_copy(qt_r[64:128, :], qt_ps)
                return qt_r

            def new_slab(k):
                return {
                    "at": atpool.tile([128, MCHUNKS, SLAB], BF16, tag="at", name=f"at{k}"),
                    "u": upsum.tile([65, SLAB], F32, tag="u", name=f"u{k}"),
                    "s": [
                        spsum.tile([128, 3, SLAB], F32, tag="s", name=f"s{k}_{g}")
                        for g in range(len(GROUPS))
                    ],
                }

            def mm1(st, qt, chunks, paired):
                for h, c in enumerate(chunks):
                    g, j = _chunk_slot(c)
                    lo = 64 * h if (paired and ROWTILE) else 0
                    nc.tensor.matmul(
                        st["s"][g][:, j, :],
                        lhsT=memT_r[lo : lo + 64, 128 * c : 128 * (c + 1)],
                        rhs=qt[lo : lo + 64, :],
                        start=True,
                        stop=True,
                        tile_position=(lo, 0) if ROWTILE else None,
                    )

            def exp_group(st, g):
                c0, cn = GROUPS[g]
                nc.scalar.activation(
                    st["at"][:, c0 : c0 + cn, :], st["s"][g][:, 0:cn, :], EXP
                )

            def mm2_group(st, g):
                c0, cn = GROUPS[g]
                for j in range(cn):
                    c = c0 + j
                    nc.tensor.matmul(
                        st["u"],
                        lhsT=maug[:, c, :],
                        rhs=st["at"][:, c, :],
                        start=(c == 0),
                        stop=(c == MCHUNKS - 1),
                    )

            def epilogue(st, i):
                u_sb = uspool.tile([65, SLAB], F32, tag="us", name=f"us{i}")
                nc.vector.tensor_copy(u_sb, st["u"])
                # o_ps shares the qt staging bank (disjoint lifetimes)
                o_ps = qtpsum.tile([128, NT, D + 1], F32, tag="qt", name=f"o{i}")
                for t in range(NT):
                    nc.tensor.transpose(
                        o_ps[:, t, :],
                        u_sb[:, 128 * t : 128 * (t + 1)],
                        ident[0:65, 0:65],
                    )
                r_sb = rpool.tile([128, NT], F32, tag="r", name=f"r{i}")
                nc.vector.reciprocal(r_sb, o_ps[:, :, D])
                o_sb = opool.tile([128, NT, D], F32, tag="o", name=f"o_sb{i}")
                for t in range(NT):
                    nc.vector.tensor_scalar_mul(
                        o_sb[:, t, :], o_ps[:, t, 0:D], r_sb[:, t : t + 1]
                    )
                nc.sync.dma_start(out=ov[i], in_=o_sb)

            def full_pass():
                # Software pipeline: slab k's group-C matmul-2 and epilogue are
                # emitted inside iteration k+1, after slab k+1's first mm1
                # group, so the in-order PE queue never puts exp-blocked work
                # in front of the matmuls the ACT engine is waiting on.
                qt = {0: load_qt(0)}
                st = {}
                for k in range(nslabs):
                    st[k] = new_slab(k)
                    # group-A mm1: pair (0,1) early, chunk 2 solo
                    mm1(st[k], qt[k], (0, 1), paired=True)
                    mm1(st[k], qt[k], (2,), paired=False)
                    if k + 1 < nslabs:
                        qt[k + 1] = load_qt(k + 1)
                    if k > 0:
                        mm2_group(st[k - 1], 2)
                        epilogue(st[k - 1], k - 1)
                        del st[k - 1]
                    exp_group(st[k], 0)
                    mm2_group(st[k], 0)
                    # group-B mm1: chunk 3 solo (waits exp-C(k-1) bank),
                    # then pair (4,5); group-C pair (6,7) right after
                    mm1(st[k], qt[k], (3,), paired=False)
                    mm1(st[k], qt[k], (4, 5), paired=True)
                    mm1(st[k], qt[k], (6, 7), paired=True)
                    exp_group(st[k], 1)
                    mm2_group(st[k], 1)
                    exp_group(st[k], 2)
                    del qt[k]
                last = nslabs - 1
                mm2_group(st[last], 2)
                epilogue(st[last], last)

            if repeat > 1:
                with tc.For_i(0, repeat, 1):
                    full_pass()
            else:
                full_pass()


def build_bass(n_loc: int = NLOC, repeat: int = 1) -> bacc.Bacc:
    nc = bacc.Bacc("TRN2", target_bir_lowering=False, debug=False)
    q = nc.dram_tensor("query", [n_loc, D], F32, kind="ExternalInput").ap()
    mem = nc.dram_tensor("memory", [M, D], F32, kind="ExternalInput").ap()
    out = nc.dram_tensor("out", [n_loc, D], F32, kind="ExternalOutput").ap()
    with tile.TileContext(nc) as tc:
        _body(tc, out, q, mem, repeat)
    nc.compile()
    return nc


_NC_CACHE: dict[tuple[int, int], bacc.Bacc] = {}


def _get_nc(n_loc: int = NLOC, repeat: int = 1) -> bacc.Bacc:
    key = (n_loc, repeat)
    if key not in _NC_CACHE:
        _NC_CACHE[key] = build_bass(n_loc, repeat)
    return _NC_CACHE[key]


def run(query: np.ndarray, memory: np.ndarray, repeat: int = 1, **spmd_kwargs):
    """Run on 8 cores; returns (out [N, 64], BassKernelResults)."""
    query = np.ascontiguousarray(np.asarray(query, dtype=np.float32))
    memory = np.ascontiguousarray(np.asarray(memory, dtype=np.float32))
    assert query.shape == (N, D) and memory.shape == (M, D)
    nc = _get_nc(NLOC, repeat)
    in_maps = [
        {"query": query[i * NLOC : (i + 1) * NLOC], "memory": memory}
        for i in range(NCORES)
    ]
    res = run_bass_kernel_spmd(nc, in_maps, list(range(NCORES)), **spmd_kwargs)
    out = np.concatenate([res.results[i]["out"] for i in range(NCORES)], axis=0)
    return out, res


def kernel(query: np.ndarray, memory: np.ndarray) -> np.ndarray:
    out, _ = run(query, memory)
    return out


# revision 12
# speedup vs baseline: 1.7469x; 1.5107x over previous
"""Trainium2 Bass kernel for the MemoryModule problem.

Computes, for query [N, 64] and memory [1024, 64]:
    out = softmax(query @ memory.T, axis=-1) @ memory

Data-parallel across 8 NeuronCores: query sharded along N, memory
replicated.  Per-core plan (N_loc = 32768 rows, slabs of 512):

  1. DMA q slab [512, 64] -> SBUF, PE-transpose to qT [64, 512],
     replicated into both partition halves [128, 512] (fp16).  The qT
     chain for slab k+1 is issued at the top of slab k so it hides under
     slab k's exps (the ACT engine is the kernel's bottleneck).
  2. mm1 (fp16, 2x row-tiled): memory-chunk pairs run concurrently in
     the two 64-row halves of the PE array (K=64), producing S^T chunks
     [128, 512] in PSUM (3 bank-groups of <=3 chunks, double buffered).
  3. One Exp activation per group, PSUM -> SBUF bf16 (A^T), unnormalized.
  4. mm2 (bf16): U^T [65, 512] += [mem_c | 1].T @ A^T[c], accumulated over
     all 8 chunks in one PSUM bank.  Row 64 is the softmax denominator.
  5. PE-transpose U^T back to [n, d] chunks, DVE reciprocal + scale,
     DMA out.

The softmax skips max-subtraction: |S| <= |q||m| <= ~45 for these inputs
(randn, D=64), exp fits comfortably in fp32/bf16 range.
"""

import numpy as np

import concourse.bass as bass
import concourse.tile as tile
from concourse import bacc, mybir
from concourse.bass_utils import run_bass_kernel_spmd
from concourse.masks import make_identity

N, M, D = 262144, 1024, 64
NCORES = 8
NLOC = N // NCORES
SLAB = 512
NT = SLAB // 128  # 128-row tiles per slab
MCHUNKS = M // 128  # 8
GROUPS = [(0, 3), (3, 3), (6, 2)]  # (first chunk, n chunks) per exp group
ROWTILE = True  # run memory-chunk pairs concurrently in PE row halves

F32 = mybir.dt.float32
F32R = mybir.dt.float32r
F16 = mybir.dt.float16
BF16 = mybir.dt.bfloat16
EXP = mybir.ActivationFunctionType.Exp


def _chunk_slot(c):
    for g, (c0, cn) in enumerate(GROUPS):
        if c0 <= c < c0 + cn:
            return g, c - c0
    raise ValueError(c)


def _body(tc: tile.TileContext, out: bass.AP, q: bass.AP, mem: bass.AP, repeat: int):
    nc = tc.nc
    n_loc = q.shape[0]
    nslabs = n_loc // SLAB

    with (
        tc.tile_pool(name="const", bufs=1) as const,
        tc.tile_pool(name="qpool", bufs=2) as qpool,
        tc.tile_pool(name="qtpool", bufs=2) as qtpool,
        tc.tile_pool(name="atpool", bufs=2) as atpool,
        tc.tile_pool(name="uspool", bufs=2) as uspool,
        tc.tile_pool(name="rpool", bufs=2) as rpool,
        tc.tile_pool(name="opool", bufs=3) as opool,
    ):
        ident = const.tile([128, 128], F32, tag="ident")
        make_identity(nc, ident)

        # touch Exp immediately so the ACT table load overlaps the preamble
        warm = const.tile([128, 2], F32, tag="warm")
        nc.vector.memset(warm, 0.0)
        nc.scalar.activation(warm[:, 1:2], warm[:, 0:1], EXP)

        # memory staged [128, chunk, d]; row m = 128*c + p
        mstage = const.tile([128, MCHUNKS, D], F32, tag="mstage")
        nc.sync.dma_start(out=mstage, in_=mem.rearrange("(c p) d -> p c d", p=128))

        # mm2 stationary: [mem_c | ones] in bf16
        maug = const.tile([128, MCHUNKS, D + 1], BF16, tag="maug")
        nc.vector.tensor_copy(maug[:, :, 0:D], mstage)
        nc.vector.memset(maug[:, :, D : D + 1], 1.0)

        # memT [d, m] via PE transposes (one-time PSUM staging pool).
        # Stored as float32r, replicated into both partition halves so
        # chunk pairs can run row-tiled in the two halves of the PE array.
        memT_r = const.tile([128, M], F16, tag="memT")
        with tc.tile_pool(name="mtps", bufs=1, space="PSUM") as mtps:
            mt_ps = mtps.tile([64, M], F32, tag="mtps")
            for c in range(MCHUNKS):
                nc.tensor.transpose(
                    mt_ps[:, 128 * c : 128 * (c + 1)], mstage[:, c, :], ident
                )
            nc.vector.tensor_copy(memT_r[0:64, :], mt_ps)
            nc.vector.tensor_copy(memT_r[64:128, :], mt_ps)

        qv = q.rearrange("(s t p) d -> s p t d", p=128, t=NT)
        ov = out.rearrange("(s t p) d -> s p t d", p=128, t=NT)

        with (
            tc.tile_pool(name="spsum", bufs=2, space="PSUM") as spsum,
            tc.tile_pool(name="qtpsum", bufs=1, space="PSUM") as qtpsum,
            tc.tile_pool(name="upsum", bufs=1, space="PSUM") as upsum,
        ):

            def load_qt(i):
                """DMA slab i's q and produce replicated qT [128, SLAB] fp16.

                Two stacked [128, 128] PE transposes (pairs of 64-wide q
                tiles side by side) instead of four [128, 64] ones — PE
                transposes are latency-dominated on HW.  The stacked output
                holds tile 2u's qT on partitions 0-63 and tile 2u+1's on
                64-127; four strided DVE copies unstack and replicate.
                """
                q_sb = qpool.tile([128, NT, D], F32, tag="q")
                nc.sync.dma_start(out=q_sb, in_=qv[i])
                qt_ps = qtpsum.tile([128, NT // 2, 128], F32, tag="qt")
                for u in range(NT // 2):
                    nc.tensor.transpose(
                        qt_ps[:, u, :], q_sb[:, 2 * u : 2 * u + 2, :], ident
                    )
                qt_r = qtpool.tile([128, SLAB], F16, tag="qts")
                for dst in (qt_r[0:64, :], qt_r[64:128, :]):
                    dv = dst.rearrange("p (u x n) -> p u x n", u=NT // 2, x=2)
                    nc.vector.tensor_copy(dv[:, :, 0, :], qt_ps[0:64, :, :])
                    nc.vector.tensor_copy(dv[:, :, 1, :], qt_ps[64:128, :, :])
                return qt_r

            def new_slab(k):
                return {
                    "at": atpool.tile([128, MCHUNKS, SLAB], BF16, tag="at", name=f"at{k}"),
                    "u": upsum.tile([65, SLAB], F32, tag="u", name=f"u{k}"),
                    "s": [
                        spsum.tile([128, 3, SLAB], F32, tag="s", name=f"s{k}_{g}")
                        for g in range(len(GROUPS))
                    ],
                }

            def mm1(st, qt, chunks, paired):
                for h, c in enumerate(chunks):
                    g, j = _chunk_slot(c)
                    lo = 64 * h if (paired and ROWTILE) else 0
                    nc.tensor.matmul(
                        st["s"][g][:, j, :],
                        lhsT=memT_r[lo : lo + 64, 128 * c : 128 * (c + 1)],
                        rhs=qt[lo : lo + 64, :],
                        start=True,
                        stop=True,
                        tile_position=(lo, 0) if ROWTILE else None,
                    )

            def exp_group(st, g):
                c0, cn = GROUPS[g]
                nc.scalar.activation(
                    st["at"][:, c0 : c0 + cn, :], st["s"][g][:, 0:cn, :], EXP
                )

            def mm2_group(st, g):
                c0, cn = GROUPS[g]
                for j in range(cn):
                    c = c0 + j
                    nc.tensor.matmul(
                        st["u"],
                        lhsT=maug[:, c, :],
                        rhs=st["at"][:, c, :],
                        start=(c == 0),
                        stop=(c == MCHUNKS - 1),
                    )

            def epilogue(st, i):
                u_sb = uspool.tile([65, SLAB], F32, tag="us", name=f"us{i}")
                nc.vector.tensor_copy(u_sb, st["u"])
                # o_ps shares the qt staging bank (disjoint lifetimes)
                o_ps = qtpsum.tile([128, NT, D + 1], F32, tag="qt", name=f"o{i}")
                for t in range(NT):
                    nc.tensor.transpose(
                        o_ps[:, t, :],
                        u_sb[:, 128 * t : 128 * (t + 1)],
                        ident[0:65, 0:65],
                    )
                r_sb = rpool.tile([128, NT], F32, tag="r", name=f"r{i}")
                nc.vector.reciprocal(r_sb, o_ps[:, :, D])
                o_sb = opool.tile([128, NT, D], F32, tag="o", name=f"o_sb{i}")
                for t in range(NT):
                    nc.vector.tensor_scalar_mul(
                        o_sb[:, t, :], o_ps[:, t, 0:D], r_sb[:, t : t + 1]
                    )
                nc.sync.dma_start(out=ov[i], in_=o_sb)

            def full_pass():
                # Software pipeline: slab k's group-C matmul-2 and epilogue are
                # emitted inside iteration k+1, after slab k+1's first mm1
                # group, so the in-order PE queue never puts exp-blocked work
                # in front of the matmuls the ACT engine is waiting on.
                qt = {0: load_qt(0)}
                st = {}
                for k in range(nslabs):
                    st[k] = new_slab(k)
                    # group-A mm1: pair (0,1) early, chunk 2 solo
                    mm1(st[k], qt[k], (0, 1), paired=True)
                    mm1(st[k], qt[k], (2,), paired=False)
                    if k + 1 < nslabs:
                        qt[k + 1] = load_qt(k + 1)
                    if k > 0:
                        mm2_group(st[k - 1], 2)
                        epilogue(st[k - 1], k - 1)
                        del st[k - 1]
                    exp_group(st[k], 0)
                    mm2_group(st[k], 0)
                    # group-B mm1: chunk 3 solo (waits exp-C(k-1) bank),
                    # then pair (4,5); group-C pair (6,7) right after
                    mm1(st[k], qt[k], (3,), paired=False)
                    mm1(st[k], qt[k], (4, 5), paired=True)
                    mm1(st[k], qt[k], (6, 7), paired=True)
                    exp_group(st[k], 1)
                    mm2_group(st[k], 1)
                    exp_group(st[k], 2)
                    del qt[k]
                last = nslabs - 1
                mm2_group(st[last], 2)
                epilogue(st[last], last)

            if repeat > 1:
                with tc.For_i(0, repeat, 1):
                    full_pass()
            else:
                full_pass()


def build_bass(n_loc: int = NLOC, repeat: int = 1) -> bacc.Bacc:
    nc = bacc.Bacc("TRN2", target_bir_lowering=False, debug=False)
    q = nc.dram_tensor("query", [n_loc, D], F32, kind="ExternalInput").ap()
    mem = nc.dram_tensor("memory", [M, D], F32, kind="ExternalInput").ap()
    out = nc.dram_tensor("out", [n_loc, D], F32, kind="ExternalOutput").ap()
    with tile.TileContext(nc) as tc:
        _body(tc, out, q, mem, repeat)
    nc.compile()
    return nc


_NC_CACHE: dict[tuple[int, int], bacc.Bacc] = {}


def _get_nc(n_loc: int = NLOC, repeat: int = 1) -> bacc.Bacc:
    key = (n_loc, repeat)
    if key not in _NC_CACHE:
        _NC_CACHE[key] = build_bass(n_loc, repeat)
    return _NC_CACHE[key]


def run(query: np.ndarray, memory: np.ndarray, repeat: int = 1, **spmd_kwargs):
    """Run on 8 cores; returns (out [N, 64], BassKernelResults)."""
    query = np.ascontiguousarray(np.asarray(query, dtype=np.float32))
    memory = np.ascontiguousarray(np.asarray(memory, dtype=np.float32))
    assert query.shape == (N, D) and memory.shape == (M, D)
    nc = _get_nc(NLOC, repeat)
    in_maps = [
        {"query": query[i * NLOC : (i + 1) * NLOC], "memory": memory}
        for i in range(NCORES)
    ]
    res = run_bass_kernel_spmd(nc, in_maps, list(range(NCORES)), **spmd_kwargs)
    out = np.concatenate([res.results[i]["out"] for i in range(NCORES)], axis=0)
    return out, res


def kernel(query: np.ndarray, memory: np.ndarray) -> np.ndarray:
    out, _ = run(query, memory)
    return out
